# revision 6
# baseline (speedup 1.0000x reference)
"""Trainium2 Bass kernel for nn_CustomBERTModel (topk_masking).

Reference computation:
    values, indices = top_k(logits, 16)          # [B, S, 16] over vocab 32000
    out = softmax(values @ W.T + b)              # tiny 16x16 linear
    out *= (input_ids == mask_token_id)
    result = zeros_like(logits).at[..., indices].set(out)

Strategy (8 cores, data-parallel over B*S = 4096 rows; 512 rows/core):
  Per 128-row tile:
    1. Stream logits through SBUF, chunk-max reduce -> M [128, 250]
       (250 chunks of 128 along vocab).
    2. Top-16 chunk maxima + chunk ids of M via DVE max8/max_index/
       match_replace (any top-16 element must live in a top-16 chunk).
    3. Indirect-DMA gather those 16 chunks/row from DRAM -> G [128, 2048].
    4. Top-16 of G (values + positions) via max8 family.
    5. Recover global vocab positions with a tiny one-hot select.
    6. 16x16 linear via PE (transpose + matmul), softmax via ACT exp,
       mask multiply.
    7. Indirect-DMA scatter the 16 weights/row into the (pre-zeroed)
       output at the top-k positions.

The ExternalOutput DRAM buffer is zero-initialized by the runtime
(bass2jax donates freshly-zeroed buffers; the native path pre-zeros), so
only the k nonzero values per row are written.
"""

import numpy as np

# Problem constants (hardcoded per contract - kernel.py is self-contained).
B, S, V, K = 2, 2048, 32000, 16
NCORES = 8
R = (B * S) // NCORES          # rows per core = 512
P = 128                        # partitions
NT = R // P                    # row-tiles per core = 4
L = 128                        # chunk length along vocab
NCH = V // L                   # chunks per row = 250
# Column-blocks for streaming the chunk-max reduce (in chunk units).
QSPLIT = [64, 64, 64, 58]      # 64+64+64+58 = 250 chunks
NEG = -1.0e30

_CACHE = {}


def _build():
    from contextlib import ExitStack

    import concourse.bass as bass
    import concourse.tile as tile
    from concourse import bacc, mybir
    from concourse.bass import IndirectOffsetOnAxis
    from concourse.masks import make_identity

    f32 = mybir.dt.float32
    i32 = mybir.dt.int32
    u32 = mybir.dt.uint32
    Alu = mybir.AluOpType
    Act = mybir.ActivationFunctionType
    Ax = mybir.AxisListType

    nc = bacc.Bacc("TRN2", target_bir_lowering=False, debug=False,
                   num_devices=NCORES)
    logits = nc.dram_tensor("logits", [R, V], f32, kind="ExternalInput").ap()
    ids = nc.dram_tensor("ids", [R, 1], f32, kind="ExternalInput").ap()
    wb = nc.dram_tensor("wb", [K + 1, K], f32, kind="ExternalInput").ap()
    out = nc.dram_tensor("out", [R, V], f32, kind="ExternalOutput").ap()

    logits_chunks = logits.rearrange("a (c l) -> (a c) l", l=L)
    out_flat = out.rearrange("a v -> (a v)")[:, None]

    with tile.TileContext(nc) as tc, ExitStack() as ctx:
        const = ctx.enter_context(tc.tile_pool(name="const", bufs=1))
        xpool = ctx.enter_context(tc.tile_pool(name="x", bufs=3))
        mpool = ctx.enter_context(tc.tile_pool(name="m", bufs=2))
        gpool = ctx.enter_context(tc.tile_pool(name="g", bufs=2))
        sm = ctx.enter_context(tc.tile_pool(name="sm", bufs=2))
        pt = ctx.enter_context(tc.tile_pool(name="pt", bufs=2, space="PSUM"))
        pz = ctx.enter_context(tc.tile_pool(name="pz", bufs=2, space="PSUM"))

        # --- one-time constants ---
        ident = const.tile([P, P], f32)
        make_identity(nc, ident[:])

        wbs = const.tile([K + 1, K], f32)
        nc.sync.dma_start(wbs[:], wb[:])

        # per-partition row index * NCH (chunk-table row base), as f32
        ichunk_i = const.tile([P, 1], i32)
        nc.gpsimd.iota(ichunk_i[:], pattern=[[1, 1]], base=0,
                       channel_multiplier=NCH)
        ichunk_f = const.tile([P, 1], f32)
        nc.vector.tensor_copy(ichunk_f[:], ichunk_i[:])

        # per-partition row index * V (flat output row base), as f32
        irow_i = const.tile([P, 1], i32)
        nc.gpsimd.iota(irow_i[:], pattern=[[1, 1]], base=0,
                       channel_multiplier=V)
        irow_f = const.tile([P, 1], f32)
        nc.vector.tensor_copy(irow_f[:], irow_i[:])

        # cbase[s, c] = c * L   (for the 16x16 one-hot block select)
        cbase_i = const.tile([P, K * K], i32)
        nc.gpsimd.iota(cbase_i[:], pattern=[[0, K], [L, K]], base=0,
                       channel_multiplier=0)
        cbase_f = const.tile([P, K * K], f32)
        nc.vector.tensor_copy(cbase_f[:], cbase_i[:])
        cbase2_f = const.tile([P, K * K], f32)
        nc.vector.tensor_scalar_add(cbase2_f[:], cbase_f[:], float(L))

        for t in range(NT):
            r0 = t * P

            # --- 1. streaming chunk-max reduce ---
            M = mpool.tile([P, NCH], f32)
            coff = 0
            for qc in QSPLIT:
                X = xpool.tile([P, QSPLIT[0] * L], f32, tag="xq")
                nc.sync.dma_start(
                    X[:, :qc * L],
                    logits[r0:r0 + P, coff * L:(coff + qc) * L])
                nc.vector.reduce_max(
                    M[:, coff:coff + qc],
                    X[:, :qc * L].rearrange("p (c l) -> p c l", l=L),
                    axis=Ax.X)
                coff += qc

            # --- 2. top-16 chunks of M ---
            T = sm.tile([P, 2 * 8], f32)
            Cu = sm.tile([P, 2 * 8], u32)
            nc.vector.max(T[:, 0:8], M[:])
            nc.vector.max_index(Cu[:, 0:8], T[:, 0:8], M[:])
            nc.vector.match_replace(M[:], T[:, 0:8], M[:], NEG)
            nc.vector.max(T[:, 8:16], M[:])
            nc.vector.max_index(Cu[:, 8:16], T[:, 8:16], M[:])

            C_f = sm.tile([P, K], f32)
            nc.vector.tensor_copy(C_f[:], Cu[:])

            # global chunk-table index = row*NCH + C  (row = r0 + p)
            GCf = sm.tile([P, K], f32)
            nc.vector.scalar_tensor_tensor(
                out=GCf[:], in0=C_f[:], scalar=float(r0 * NCH), op0=Alu.add,
                in1=ichunk_f[:].to_broadcast([P, K]), op1=Alu.add)
            GCu = sm.tile([P, K], u32)
            nc.vector.tensor_copy(GCu[:], GCf[:])

            # --- 3. gather candidate chunks from DRAM ---
            # HW indirect DMA: one offset per partition per instruction;
            # each descriptor moves the dest free extent contiguously.
            G = gpool.tile([P, K * L], f32)
            for r in range(K):
                nc.gpsimd.indirect_dma_start(
                    out=G[:, r * L:(r + 1) * L], out_offset=None,
                    in_=logits_chunks[:],
                    in_offset=IndirectOffsetOnAxis(ap=GCu[:, r:r + 1], axis=0))

            # --- 4. top-16 of G ---
            Vv = sm.tile([P, 2 * 8], f32)
            Ju = sm.tile([P, 2 * 8], u32)
            nc.vector.max(Vv[:, 0:8], G[:])
            nc.vector.max_index(Ju[:, 0:8], Vv[:, 0:8], G[:])
            nc.vector.match_replace(G[:], Vv[:, 0:8], G[:], NEG)
            nc.vector.max(Vv[:, 8:16], G[:])
            nc.vector.max_index(Ju[:, 8:16], Vv[:, 8:16], G[:])

            # --- 5. global positions ---
            J_f = sm.tile([P, K], f32)
            nc.vector.tensor_copy(J_f[:], Ju[:])

            # one-hot over the 16 candidate blocks: OH[s,c] = (cbase[c] <= J[s] < cbase[c]+L)
            JB = J_f[:, :, None].to_broadcast([P, K, K])
            c3 = cbase_f[:].rearrange("p (s c) -> p s c", c=K)
            c23 = cbase2_f[:].rearrange("p (s c) -> p s c", c=K)
            OH1 = sm.tile([P, K * K], f32)
            nc.vector.tensor_tensor(
                out=OH1[:].rearrange("p (s c) -> p s c", c=K),
                in0=JB, in1=c3, op=Alu.is_ge)
            OH2 = sm.tile([P, K * K], f32)
            nc.vector.tensor_tensor(
                out=OH2[:].rearrange("p (s c) -> p s c", c=K),
                in0=JB, in1=c23, op=Alu.is_lt)
            OH = sm.tile([P, K * K], f32)
            nc.vector.tensor_tensor(out=OH[:], in0=OH1[:], in1=OH2[:],
                                    op=Alu.mult)

            # DIFF[s,c] = C[c]*L - cbase[s,c]; SUMD[s] = sum_c OH*DIFF
            # global pos = J + SUMD + row*V
            CC_f = sm.tile([P, K], f32)
            nc.vector.tensor_scalar_mul(CC_f[:], C_f[:], float(L))
            DIFF = sm.tile([P, K * K], f32)
            nc.vector.tensor_tensor(
                out=DIFF[:].rearrange("p (s c) -> p s c", c=K),
                in0=CC_f[:, None, :].to_broadcast([P, K, K]),
                in1=c3, op=Alu.subtract)
            MUL = sm.tile([P, K * K], f32)
            nc.vector.tensor_tensor(out=MUL[:], in0=OH[:], in1=DIFF[:],
                                    op=Alu.mult)
            SUMD = sm.tile([P, K], f32)
            nc.vector.reduce_sum(
                SUMD[:], MUL[:].rearrange("p (s c) -> p s c", c=K), axis=Ax.X)

            A1 = sm.tile([P, K], f32)
            nc.vector.tensor_tensor(out=A1[:], in0=J_f[:], in1=SUMD[:],
                                    op=Alu.add)
            FPf = sm.tile([P, K], f32)
            nc.vector.scalar_tensor_tensor(
                out=FPf[:], in0=A1[:], scalar=float(r0 * V), op0=Alu.add,
                in1=irow_f[:].to_broadcast([P, K]), op1=Alu.add)
            FPu = sm.tile([P, K], u32)
            nc.vector.tensor_copy(FPu[:], FPf[:])

            # --- 6. linear + softmax + mask ---
            Vt = pt.tile([K, P], f32, space="PSUM")
            nc.tensor.transpose(out=Vt[:], in_=Vv[:], identity=ident[:])
            VA = sm.tile([K + 1, P], f32)
            nc.gpsimd.memset(VA[:], 1.0)
            nc.scalar.activation(VA[0:K, :], Vt[:], Act.Copy)

            Z = pz.tile([P, K], f32, space="PSUM")
            nc.tensor.matmul(Z[:], lhsT=VA[:], rhs=wbs[:], start=True,
                             stop=True)

            mx = sm.tile([P, 1], f32)
            nc.vector.reduce_max(mx[:], Z[:], axis=Ax.X, negate=True)
            E = sm.tile([P, K], f32)
            ssum = sm.tile([P, 1], f32)
            nc.scalar.activation(E[:], Z[:], Act.Exp, bias=mx[:], scale=1.0,
                                 accum_out=ssum[:])
            rec = sm.tile([P, 1], f32)
            nc.vector.reciprocal(rec[:], ssum[:])

            idt = sm.tile([P, 1], f32)
            nc.sync.dma_start(idt[:], ids[r0:r0 + P, :])
            msk = sm.tile([P, 1], f32)
            nc.vector.tensor_scalar(out=msk[:], in0=idt[:], scalar1=5.0,
                                    scalar2=None, op0=Alu.is_equal)
            wfac = sm.tile([P, 1], f32)
            nc.vector.tensor_tensor(out=wfac[:], in0=rec[:], in1=msk[:],
                                    op=Alu.mult)
            WGT = sm.tile([P, K], f32)
            nc.vector.tensor_scalar(out=WGT[:], in0=E[:], scalar1=wfac[:],
                                    scalar2=None, op0=Alu.mult)

            # --- 7. scatter the 16 weights/row ---
            for r in range(K):
                nc.gpsimd.indirect_dma_start(
                    out=out_flat[:],
                    out_offset=IndirectOffsetOnAxis(ap=FPu[:, r:r + 1], axis=0),
                    in_=WGT[:, r:r + 1], in_offset=None)

    nc.compile()
    return nc


def _get_nc():
    if "nc" not in _CACHE:
        _CACHE["nc"] = _build()
    return _CACHE["nc"]


def kernel(logits, input_ids, W, b, mask_token_id, k):
    from concourse.bass_utils import run_bass_kernel_spmd

    logits = np.asarray(logits)
    input_ids = np.asarray(input_ids)
    W = np.asarray(W, dtype=np.float32)
    b = np.asarray(b, dtype=np.float32)
    assert logits.shape == (B, S, V) and int(k) == K
    assert int(mask_token_id) == 5

    lg = np.ascontiguousarray(logits, dtype=np.float32).reshape(B * S, V)
    ids_f = (input_ids.reshape(B * S).astype(np.float32))[:, None]
    wbmat = np.concatenate([W.T, b[None, :]], axis=0).astype(np.float32)

    nc = _get_nc()
    in_maps = [
        {
            "logits": lg[c * R:(c + 1) * R],
            "ids": np.ascontiguousarray(ids_f[c * R:(c + 1) * R]),
            "wb": wbmat,
        }
        for c in range(NCORES)
    ]
    res = run_bass_kernel_spmd(nc, in_maps, list(range(NCORES))).results
    outs = np.concatenate([res[c]["out"] for c in range(NCORES)], axis=0)
    out = outs.reshape(B, S, V)
    if out.dtype != logits.dtype:
        out = out.astype(logits.dtype)
    return out
